# revision 4
# baseline (speedup 1.0000x reference)
"""GCNConv (add self-loops, symmetric norm, linear, relu, broadcast) on 8 TRN2 cores.

Sharding: destination nodes are row-sharded across the 8 cores (1250 rows each).
Each core computes the full h = x @ W (x is supplied pre-transposed and padded by
the host), writes it to its local DRAM, then for each 128-row destination tile
gathers the source-node h rows for that tile's (sorted, padded) edge list with
dma_gather and reduces them with PE matmuls against host-built block scatter
matrices S (S[e, d] = norm_e if dst_local(e) == d else 0).  Bias-add + relu on
DVE, then the [128, 301] result is expanded to [128, 301*12] with strided
copies and DMA'd 4x (head dim) to the output slab.
"""

import numpy as np

import concourse.bass as bass
import concourse.bacc as bacc
import concourse.mybir as mybir
import concourse.tile as tile
from concourse.bass_utils import run_bass_kernel_spmd

N_NODES = 10000
N_GENES = 978
EMBED = 301
HEADS = 4
REP = 12
N_CORES = 8
NPC = N_NODES // N_CORES          # 1250 dst rows per core
DT = 128                          # dst tile height
NT = (NPC + DT - 1) // DT         # 10 dst tiles per core
GP = 1024                         # padded gene dim (8 chunks of 128)
GCH = GP // 128
SP = 10112                        # padded node dim for h (79 tiles of 128)
ST = SP // 128
HROW = 320                        # h row padded to 320 f32 = 1280 B (256-aligned)

F32 = mybir.dt.float32
F32R = mybir.dt.float32r
I16 = mybir.dt.int16

_prog_cache: dict = {}


def _build_program(bmax: int, mm_dtype):
    slots = bmax * 128
    nc = bacc.Bacc("TRN2", target_bir_lowering=False, debug=False,
                   num_devices=N_CORES, num_swdge_queues=4)

    xT_d = nc.dram_tensor("xT", [GP, SP], F32, kind="ExternalInput")
    W_d = nc.dram_tensor("Wp", [GP, EMBED], F32, kind="ExternalInput")
    b_d = nc.dram_tensor("bB", [128, EMBED], F32, kind="ExternalInput")
    S_d = nc.dram_tensor("Sblk", [NT, 128, slots], F32, kind="ExternalInput")
    ix_d = nc.dram_tensor("idxw", [NT, 128, slots // 16], I16, kind="ExternalInput")
    out_d = nc.dram_tensor("out", [NPC, HEADS, EMBED, REP], F32, kind="ExternalOutput")
    h_d = nc.dram_tensor("hbuf", [SP, HROW], F32)

    def mm_ap(ap):
        return ap.bitcast(mm_dtype) if mm_dtype is not F32 else ap

    with tile.TileContext(nc) as tc:
        with tc.tile_pool(name="const", bufs=1) as cpool:
            b_sb = cpool.tile([128, EMBED], F32)
            nc.sync.dma_start(b_sb[:], b_d[:])

            # ---------------- phase 0: h = x @ W ----------------
            with (
                tc.tile_pool(name="wsb", bufs=1) as wpool,
                tc.tile_pool(name="xt", bufs=16) as xpool,
                tc.tile_pool(name="hsb", bufs=4) as hpool,
                tc.tile_pool(name="ph", bufs=4, space="PSUM") as phpool,
            ):
                w_sb = wpool.tile([128, GCH, EMBED], F32)
                for g in range(GCH):
                    nc.sync.dma_start(w_sb[:, g, :], W_d[g * 128:(g + 1) * 128, :])

                SG = 512
                for s0 in range(0, SP, SG):
                    sgw = min(SG, SP - s0)
                    xts = []
                    for g in range(GCH):
                        xt = xpool.tile([128, SG], F32, tag="xt")
                        nc.sync.dma_start(xt[:, :sgw],
                                          xT_d[g * 128:(g + 1) * 128, s0:s0 + sgw])
                        xts.append(xt)
                    for sub in range(sgw // 128):
                        ph = phpool.tile([128, EMBED], F32)
                        for g in range(GCH):
                            nc.tensor.matmul(
                                ph[:],
                                mm_ap(xts[g][:, sub * 128:(sub + 1) * 128]),
                                mm_ap(w_sb[:, g, :]),
                                start=(g == 0), stop=(g == GCH - 1),
                            )
                        h_sb = hpool.tile([128, EMBED], F32)
                        nc.vector.tensor_copy(h_sb[:], ph[:])
                        nc.sync.dma_start(
                            h_d[s0 + sub * 128:s0 + (sub + 1) * 128, :EMBED], h_sb[:])

            tc.strict_bb_all_engine_barrier()

            # ---------------- phase 1: aggregate + bias + relu + expand ----
            with (
                tc.tile_pool(name="sS", bufs=2) as spool,
                tc.tile_pool(name="sI", bufs=2) as ipool,
                tc.tile_pool(name="sG", bufs=2) as gpool,
                tc.tile_pool(name="sO", bufs=2) as opool,
                tc.tile_pool(name="pO", bufs=4, space="PSUM") as popool,
            ):
                for t in range(NT):
                    r0 = t * DT
                    nr = min(DT, NPC - r0)
                    s_sb = spool.tile([128, slots], F32, tag="s")
                    nc.sync.dma_start(s_sb[:], S_d[t])
                    ix_sb = ipool.tile([128, slots // 16], I16, tag="ix")
                    nc.sync.dma_start(ix_sb[:], ix_d[t])
                    g_sb = gpool.tile([128, bmax, HROW], F32, tag="g")
                    # SWDGE descriptor carveout holds 1024 descriptors per
                    # queue — split the tile's gather into <=8-block chunks
                    # (1024 rows) rotated across the 4 SWDGE queues.
                    GBLK = 8
                    for gi, b0 in enumerate(range(0, bmax, GBLK)):
                        nb = min(GBLK, bmax - b0)
                        nc.gpsimd.dma_gather(
                            g_sb[:, b0:b0 + nb, :], h_d[:],
                            ix_sb[:, b0 * 8:(b0 + nb) * 8],
                            num_idxs=nb * 128, num_idxs_reg=nb * 128,
                            elem_size=HROW, queue_num=gi % 4,
                        )
                    po = popool.tile([128, EMBED], F32)
                    for blk in range(bmax):
                        nc.tensor.matmul(
                            po[:],
                            mm_ap(s_sb[:, blk * 128:(blk + 1) * 128]),
                            mm_ap(g_sb[:, blk, :EMBED]),
                            start=(blk == 0), stop=(blk == bmax - 1),
                        )
                    o_sm = opool.tile([128, EMBED], F32, tag="osm")
                    nc.vector.tensor_add(o_sm[:], po[:], b_sb[:])
                    nc.vector.tensor_relu(o_sm[:], o_sm[:])
                    o_rep = opool.tile([128, EMBED * REP], F32, tag="orep")
                    o_rv = o_rep.rearrange("p (j r) -> p j r", r=REP)
                    for r in range(REP):
                        nc.vector.tensor_copy(o_rv[:, :, r], o_sm[:])
                    for hh in range(HEADS):
                        nc.sync.dma_start(out_d[r0:r0 + nr, hh, :, :],
                                          o_rep[:nr, :])

    nc.compile()
    return nc


def _preprocess(x, edge_index, edge_weight, W, b):
    src = np.concatenate([edge_index[0].astype(np.int64),
                          np.arange(N_NODES, dtype=np.int64)])
    dst = np.concatenate([edge_index[1].astype(np.int64),
                          np.arange(N_NODES, dtype=np.int64)])
    wf = np.concatenate([edge_weight.astype(np.float32),
                         np.ones(N_NODES, np.float32)])

    deg = np.bincount(dst, weights=wf.astype(np.float64),
                      minlength=N_NODES).astype(np.float32)
    dis = np.where(deg > 0, 1.0 / np.sqrt(deg), 0.0).astype(np.float32)
    norm = (dis[src] * wf * dis[dst]).astype(np.float32)

    order = np.argsort(dst, kind="stable")
    src_s, dst_s, norm_s = src[order], dst[order], norm[order]

    core_of = dst_s // NPC
    tloc_of = (dst_s % NPC) // DT
    group = core_of * NT + tloc_of
    cnt = np.bincount(group, minlength=N_CORES * NT)
    bmax = int(np.ceil(cnt.max() / 128))
    slots = bmax * 128

    gstart = np.zeros(N_CORES * NT + 1, np.int64)
    gstart[1:] = np.cumsum(cnt)
    slot = np.arange(len(group)) - gstart[group]
    kk = group // NT
    tt = group % NT
    dloc = (dst_s % NPC) % DT

    idx_arr = np.zeros((N_CORES, NT, slots), np.int16)
    idx_arr[kk, tt, slot] = src_s.astype(np.int16)
    S_arr = np.zeros((N_CORES, NT, 128, slots), np.float32)
    S_arr[kk, tt, slot % 128, (slot // 128) * 128 + dloc] = norm_s

    # SWDGE index layout: idx i lives at (partition i%16, col i//16),
    # replicated across the 8 sixteen-partition groups.
    cols = np.arange(slots // 16)
    idx_w = np.empty((N_CORES, NT, 128, slots // 16), np.int16)
    base = idx_arr[:, :, cols * 16]
    for p in range(16):
        lane = idx_arr[:, :, cols * 16 + p]
        idx_w[:, :, p::16, :] = lane[:, :, None, :]

    xT = np.zeros((GP, SP), np.float32)
    xT[:N_GENES, :N_NODES] = np.ascontiguousarray(x.astype(np.float32).T)
    Wp = np.zeros((GP, EMBED), np.float32)
    Wp[:N_GENES] = W.astype(np.float32)
    bB = np.broadcast_to(b.astype(np.float32), (128, EMBED)).copy()
    return xT, Wp, bB, S_arr, idx_w, bmax


def kernel(x, edge_index, edge_weight, W, b):
    x = np.asarray(x)
    edge_index = np.asarray(edge_index)
    edge_weight = np.asarray(edge_weight)
    W = np.asarray(W)
    b = np.asarray(b)

    xT, Wp, bB, S_arr, idx_w, bmax = _preprocess(x, edge_index, edge_weight, W, b)

    mm_dtype = F32
    key = (bmax, str(mm_dtype))
    if key not in _prog_cache:
        _prog_cache[key] = _build_program(bmax, mm_dtype)
    nc = _prog_cache[key]

    in_maps = [
        {"xT": xT, "Wp": Wp, "bB": bB,
         "Sblk": S_arr[k], "idxw": idx_w[k]}
        for k in range(N_CORES)
    ]
    res = run_bass_kernel_spmd(nc, in_maps, core_ids=list(range(N_CORES)))
    out = np.concatenate([res.results[k]["out"] for k in range(N_CORES)], axis=0)
    return out
